# revision 16
# baseline (speedup 1.0000x reference)
"""Low-rank RNN Euler step on 8 Trainium2 NeuronCores.

Math (per reference):
    ff  = input @ W_in.T                      [B, H]
    C   = (L @ R) / H                         [H, H]   (never materialized)
    nh  = 0.9 h + DT*(ff + tanh(h) @ R.T @ L.T / H)
    out = tanh(nh) @ W_out.T / H              [B, O]

Sharding: data-parallel over batch, 512 -> 8 cores x 64 rows. Parameters are
replicated and pre-transposed/stacked on the host (host prep and the final
un-transpose are not part of HW exec time). On-device the only transposes are
of the hidden state, done as PE transpose-matmuls.

Hardware findings baked into the structure (all verified by probes):
  - only ONE open PSUM accumulation group per 2KB bank at a time; start=True
    pends-zero the whole bank, so each bank region is written exactly once
    and engines read the raw bytes
  - densely alternating transpose-mode and normal matmuls into one bank
    crashes the device (sim-clean), so the hT term is saved to SBUF in
    phase 1 instead of re-transposed into the G accumulation
  - DVE ops with TWO PSUM operands fail at runtime; PSUM+SBUF is fine
  - transpose-mode rhs must be a true permutation matrix (no scaled identity)
  - 8 total DMAs: one per HW lane, so no lane-FIFO coupling

Per-core dataflow (H-major middle, 64 H-tiles of 128, 8 banks of 8 tiles):
  phase1: per bank: 8 transpose-matmuls h chunk -> psum [128, 8x64];
          ACT tanh(bank) -> thT_sb; DVE 0.9*bank -> h9T_sb
  u:      64 accumulating matmuls  uT[8,64] += RT_t.T @ thT_t  (PSUM
          partitions 32..39, one long group); ACT scale-copy -> G rows 32..39
  phase2: per bank: 8 matmuls WLT_t.T @ G -> psum; DVE psum+h9T_sb -> nh_sb
          (= nh.T); ACT tanh -> th2T_sb; DMA 2 banks out (H-major,
          256B-contiguous runs)
  out:    64 accumulating matmuls outT[4,64] += WoT_t.T @ th2T_t; scale 1/H

Outputs are stored transposed ([H,64] / [O,64]) and un-transposed on host.
"""

import os
import numpy as np

B, H, IN, O, RK = 512, 8192, 16, 4, 8
DT = 0.1
NCORES = 8
BS = B // NCORES          # 64 batch rows per core
NT = H // 128             # 64 H-tiles
TPB = 8                   # H-tiles per PSUM bank
NB = NT // TPB            # 8 bank rounds

# free-axis offsets inside the small-params SBUF tile [128, PF]
OFF_EYE = 0               # [128, 64]: I64 in rows 0-63 AND rows 64-127
OFF_G = 136               # [128, 64]: G (rows 0-15 DT*x.T;
                          #            rows 32-39 written on device)
OFF_RT = 200              # [128, 512]: RT layout [p, 8t+r] = R[r, 128t+p]
OFF_WOT = 712             # [128, 256]: WoT layout [p, 4t+o] = W_out[o, 128t+p]
PF = 968

_cache = {}


def _build_nc():
    import concourse.bacc as bacc
    import concourse.tile as tile
    import concourse.mybir as mybir

    f32 = mybir.dt.float32
    AF = mybir.ActivationFunctionType

    nc = bacc.Bacc()

    sp_d = nc.dram_tensor("sp", [128, PF], f32, kind="ExternalInput")
    wlt_d = nc.dram_tensor("wlt", [40, H], f32, kind="ExternalInput")
    h_d = nc.dram_tensor("h", [128, H // 2], f32, kind="ExternalInput")
    nh_d = nc.dram_tensor("nh", [H, BS], f32, kind="ExternalOutput")
    o_d = nc.dram_tensor("o", [O, BS], f32, kind="ExternalOutput")

    with tile.TileContext(nc) as tc:
        with (
            tc.tile_pool(name="const", bufs=1) as constp,
            tc.tile_pool(name="big", bufs=1) as bigp,
            tc.tile_pool(name="ps1", bufs=3, space="PSUM") as ps1,
            tc.tile_pool(name="psu", bufs=1, space="PSUM") as psu,
            tc.tile_pool(name="ps2", bufs=3, space="PSUM") as ps2,
            tc.tile_pool(name="pso", bufs=1, space="PSUM") as pso,
        ):
            sp = constp.tile([128, PF], f32)
            wlt = constp.tile([40, H], f32)
            h2 = bigp.tile([128, H // 2], f32)
            thT = bigp.tile([128, NT * BS], f32)
            h9T = bigp.tile([128, NT * BS], f32)
            th2T = bigp.tile([128, NT * BS], f32)
            nh_sb = bigp.tile([128, NT * BS], f32)
            o_sb = constp.tile([O, BS], f32)

            # one DMA per HW lane: q0 small params, q1 wlt, q2 h
            nc.sync.dma_start(sp[:], sp_d[:])
            nc.sync.dma_start(wlt[:], wlt_d[:])
            nc.sync.dma_start(h2[:], h_d[:])

            eye = sp[:, OFF_EYE:OFF_EYE + BS]
            g2 = sp[:, OFF_G:OFF_G + BS]
            rt = sp[:, OFF_RT:OFF_RT + NT * RK]
            wot = sp[:, OFF_WOT:OFF_WOT + NT * O]

            pu = psu.tile([128, BS], f32)

            def hchunk(t):
                base = 64 * (t // 32)
                c = 128 * (t % 32)
                return h2[base:base + 64, c:c + 128], base

            # phase 1: hT tiles via PE transpose; tanh and 0.9x straight out
            # of PSUM; then this bank's slice of the u accumulation
            for b in range(NB):
                pt = ps1.tile([128, TPB * BS], f32)
                for j in range(TPB):
                    t = b * TPB + j
                    hc, base = hchunk(t)
                    nc.tensor.matmul(
                        pt[:, j * BS:(j + 1) * BS],
                        hc,
                        eye[base:base + 64, :],
                        is_transpose=True,
                        tile_position=(base, 0),
                        skip_group_check=True,
                    )
                sl = slice(b * 512, (b + 1) * 512)
                nc.scalar.activation(thT[:, sl], pt[:], AF.Tanh)
                nc.vector.tensor_scalar_mul(h9T[:, sl], pt[:], 1.0 - DT)
                for j in range(TPB):
                    t = b * TPB + j
                    nc.tensor.matmul(
                        pu[32:40, :],
                        rt[:, t * RK:(t + 1) * RK],
                        thT[:, t * BS:(t + 1) * BS],
                        start=(t == 0),
                        stop=(t == NT - 1),
                        tile_position=(0, 32),
                    )

            # G rows 32..39 = (DT/H) * uT  (rows 0..15 = DT * x.T from host)
            nc.scalar.activation(
                g2[32:40, :], pu[32:40, :], AF.Copy, scale=DT / H
            )

            po = pso.tile([O, BS], f32)

            # phase 2: psum = DT*(ff+lr).T per tile; nh.T = psum + 0.9 hT
            for b in range(NB):
                pa = ps2.tile([128, TPB * BS], f32)
                for j in range(TPB):
                    t = b * TPB + j
                    nc.tensor.matmul(
                        pa[:, j * BS:(j + 1) * BS],
                        wlt[0:40, 128 * t:128 * (t + 1)],
                        g2[0:40, :],
                        tile_position=(0, 0),
                        skip_group_check=True,
                    )
                sl = slice(b * 512, (b + 1) * 512)
                nc.vector.tensor_tensor(
                    nh_sb[:, sl], pa[:], h9T[:, sl], mybir.AluOpType.add
                )
                nc.scalar.activation(th2T[:, sl], nh_sb[:, sl], AF.Tanh)
                if b % 2 == 1:
                    # two banks per store: 3 input + 4 nh + 1 out DMA = 8
                    # total, one per HW lane
                    nc.sync.dma_start(
                        nh_d.rearrange("(t p) c -> p t c", p=128)[:, (b - 1) * TPB:(b + 1) * TPB, :],
                        nh_sb[:, (b - 1) * 512:(b + 1) * 512].rearrange("p (t c) -> p t c", c=BS),
                    )
                for j in range(TPB):
                    t = b * TPB + j
                    nc.tensor.matmul(
                        po[:],
                        wot[:, t * O:(t + 1) * O],
                        th2T[:, t * BS:(t + 1) * BS],
                        start=(t == 0),
                        stop=(t == NT - 1),
                        tile_position=(0, 0),
                    )

            nc.scalar.activation(o_sb[:], po[:], AF.Copy, scale=1.0 / H)
            nc.sync.dma_start(o_d[:], o_sb[:])

    nc.compile()
    return nc


def _prep_host_inputs(input, hidden_state, W_in, L, R, W_out):
    f = np.float32

    sp_base = np.zeros((128, PF), f)
    sp_base[0:64, OFF_EYE:OFF_EYE + BS] = np.eye(BS, dtype=f)
    sp_base[64:128, OFF_EYE:OFF_EYE + BS] = np.eye(BS, dtype=f)
    sp_base[:, OFF_RT:OFF_RT + NT * RK] = (
        R.T.reshape(NT, 128, RK).transpose(1, 0, 2).reshape(128, NT * RK)
    )
    sp_base[:, OFF_WOT:OFF_WOT + NT * O] = (
        W_out.T.reshape(NT, 128, O).transpose(1, 0, 2).reshape(128, NT * O)
    )

    wlt = np.zeros((40, H), f)
    wlt[0:IN] = W_in.T
    wlt[32:40] = L.T

    in_maps = []
    for c in range(NCORES):
        sl = slice(c * BS, (c + 1) * BS)
        sp = sp_base.copy()
        sp[0:IN, OFF_G:OFF_G + BS] = np.float32(DT) * input[sl].T
        h2 = np.ascontiguousarray(
            np.concatenate(
                [hidden_state[sl, 0:H // 2], hidden_state[sl, H // 2:]], axis=0
            )
        )
        in_maps.append({"sp": sp, "wlt": wlt, "h": h2})
    return in_maps


def kernel(input, hidden_state, W_in, L, R, W_out):
    from concourse.bass_utils import run_bass_kernel_spmd

    input = np.asarray(input, np.float32)
    hidden_state = np.asarray(hidden_state, np.float32)
    W_in = np.asarray(W_in, np.float32)
    L = np.asarray(L, np.float32)
    R = np.asarray(R, np.float32)
    W_out = np.asarray(W_out, np.float32)

    if "nc" not in _cache:
        _cache["nc"] = _build_nc()
    nc = _cache["nc"]

    in_maps = _prep_host_inputs(input, hidden_state, W_in, L, R, W_out)
    res = run_bass_kernel_spmd(
        nc,
        in_maps,
        core_ids=list(range(NCORES)),
        trace=bool(int(os.environ.get("KERNEL_TRACE", "0"))),
    )
    kernel._last_results = res

    output = np.concatenate([r["o"].T for r in res.results], axis=0)
    new_hidden = np.concatenate([r["nh"].T for r in res.results], axis=0)
    return output, new_hidden
